# revision 13
# baseline (speedup 1.0000x reference)
"""Bahdanau additive attention on 8 Trainium2 NeuronCores.

Data-parallel over batch: core c handles batches [4c, 4c+4).
Per batch b (all big PE operands bf16; PSUM accumulates fp32):
  ep[k,t]   = sum_h Ua[k,h] * enc[b,t,h]        (PE matmuls, k on PSUM partitions)
  z[k,t]    = tanh(ep[k,t] + hp[b,k])           (ScalarE, hp as per-partition bias)
  e[t]      = sum_k va[k] * z[k,t]              (M=1 PE matmuls + mask add)
  attn      = exp(e) / sum(exp(e))              (per-chunk exp straight from PSUM;
                                                 no max pass: |e| <= sum|va| ~ 26;
                                                 mask folds in as -1e30 -> exp -> 0)
  ctx[h]    = sum_t attn[t] * enc[b,t,h]        (DVE tensor_tensor_reduce against a
                                                 PE partition-broadcast of attn,
                                                 reusing the already-resident encT
                                                 tiles -- no second enc load)
hp[b,k] = sum_h Wa[k,h] * h_t[b,h] on the PE via transposed-layout matmuls.
Prologue DMAs fan out over three queues (sync/gpsimd/vector) and the first
tiles are split per h-slice so the first matmul starts ~1.5us in.
"""

import numpy as np

import concourse.bass as bass
import concourse.tile as tile
from concourse import bacc, mybir

dt = mybir.dt
AF = mybir.ActivationFunctionType
ALU = mybir.AluOpType

B, T, H = 32, 1024, 1024
NCORES = 8
BL = B // NCORES          # batches per core
P = 128                   # partitions
NT = 512                  # matmul free-dim chunk (one PSUM bank of fp32)
KT = H // P               # k-tiles (output rows of ep)
HT = H // P               # h-tiles (contraction)
TT = T // P               # t-tiles
TC = T // NT              # t chunks per batch

_CACHE = {}


def _build_nc():
    nc = bacc.Bacc("TRN2", target_bir_lowering=False, debug=False)

    # Block layouts (host-prepped) so each DMA is one contiguous transfer
    # delivering exactly what one matmul group consumes:
    #   encT: [BL, TC, H, NT]  tc-major blocks of enc[b].T
    #   uaT:  [KT, H, P]       kt-major column blocks of Ua.T
    #   waT:  [TC, H, NT]      kc-major column blocks of Wa.T
    encT_d = nc.dram_tensor("encT", [BL, TC, P, HT, NT], dt.bfloat16,
                            kind="ExternalInput").ap()
    uaT_d = nc.dram_tensor("uaT", [KT, P, HT, P], dt.bfloat16,
                           kind="ExternalInput").ap()
    waT_d = nc.dram_tensor("waT", [TC, P, HT, NT], dt.bfloat16,
                           kind="ExternalInput").ap()
    htT_d = nc.dram_tensor("htT", [H, BL], dt.bfloat16, kind="ExternalInput").ap()
    va_d = nc.dram_tensor("va", [H], dt.bfloat16, kind="ExternalInput").ap()
    mask_d = nc.dram_tensor("mask", [BL, T], dt.uint8, kind="ExternalInput").ap()

    # ctx stored [BL, P, HT] (partition-major) so the output DMA is one
    # contiguous 2D transfer; the host undoes the permutation.
    ctx_d = nc.dram_tensor("ctx", [BL, P, HT], dt.float32,
                           kind="ExternalOutput").ap()
    attn_d = nc.dram_tensor("attn", [BL, T], dt.float32, kind="ExternalOutput").ap()

    with tile.TileContext(nc) as tc:
        from contextlib import ExitStack

        with ExitStack() as st:
            wpool = st.enter_context(tc.tile_pool(name="weights", bufs=1))
            etpool = st.enter_context(tc.tile_pool(name="encT", bufs=4))
            thpool = st.enter_context(tc.tile_pool(name="tanh", bufs=12))
            scpool = st.enter_context(tc.tile_pool(name="scr", bufs=2))
            smpool = st.enter_context(tc.tile_pool(name="small", bufs=1))
            pmain = st.enter_context(tc.tile_pool(name="pmain", bufs=4, space="PSUM"))
            pe_ps = st.enter_context(tc.tile_pool(name="pe", bufs=2, space="PSUM"))
            pbc = st.enter_context(tc.tile_pool(name="pbc", bufs=2, space="PSUM"))

            # ---- constants / small inputs, fanned across DMA queues ----
            va_sb = wpool.tile([P, KT], dt.bfloat16, tag="va")
            htT_sb = wpool.tile([P, HT, BL], dt.bfloat16, tag="htT")
            nc.gpsimd.dma_start(htT_sb[:], htT_d.rearrange("(ht p) b -> p ht b", p=P))

            uaT_sb = [None] * KT

            def load_uaT(kt, split=False):
                u = wpool.tile([P, HT, P], dt.bfloat16, tag=f"uaT{kt}",
                               name=f"uaT{kt}")
                if split:
                    for ht in range(HT):
                        nc.scalar.dma_start(u[:, ht, :], uaT_d[kt, :, ht, :])
                else:
                    nc.scalar.dma_start(u[:], uaT_d[kt])
                uaT_sb[kt] = u

            def load_encT(bi, tcc, split=False):
                t_ = etpool.tile([P, HT, NT], dt.bfloat16, tag="encT",
                                 name=f"encT{bi}_{tcc}")
                if split:
                    for ht in range(HT):
                        nc.sync.dma_start(t_[:, ht, :], encT_d[bi, tcc, :, ht, :])
                else:
                    nc.sync.dma_start(t_[:], encT_d[bi, tcc])
                return t_

            def load_waT(kc):
                w = wpool.tile([P, HT, NT], dt.bfloat16, tag=f"waT{kc}",
                               name=f"waT{kc}")
                hh = HT // 2
                nc.gpsimd.dma_start(w[:, :hh, :], waT_d[kc, :, :hh, :])
                nc.gpsimd.dma_start(w[:, hh:, :], waT_d[kc, :, hh:, :])
                return w

            # First tiles feed the PE as fast as the ~600ns/DMA floor
            # allows: uaT0 unsplit on scalar, encT(0,0) slices round-robined
            # over sync/gpsimd/scalar so slices land ~every 200ns.
            load_uaT(0)
            t00 = etpool.tile([P, HT, NT], dt.bfloat16, tag="encT",
                              name="encT0_0")
            eng_ring = [nc.sync, nc.gpsimd, nc.scalar]
            for ht in range(HT):
                eng_ring[ht % 3].dma_start(t00[:, ht, :],
                                           encT_d[0, 0, :, ht, :])
            encT_b0 = [t00]
            for kt in range(1, KT):
                load_uaT(kt)
            nc.scalar.dma_start(va_sb[:], va_d.rearrange("(kt p) -> p kt", p=P))
            waT_kc = [load_waT(0), load_waT(1)]
            encT_b0.append(load_encT(0, 1))

            ones_sb = wpool.tile([1, 1], dt.float32, tag="ones")
            nc.vector.memset(ones_sb[:], 1.0)
            ones_bf = wpool.tile([1, 1], dt.bfloat16, tag="ones_bf")
            nc.vector.tensor_copy(ones_bf[:], ones_sb[:])
            onesrow_sb = wpool.tile([1, P], dt.float32, tag="onesrow")
            nc.vector.memset(onesrow_sb[:], 1.0)
            onesrow_bf = wpool.tile([1, P], dt.bfloat16, tag="onesrow_bf")
            nc.vector.tensor_copy(onesrow_bf[:], onesrow_sb[:])
            negbig = wpool.tile([1, 1], dt.float32, tag="negbig")
            nc.vector.memset(negbig[:], -1e30)
            hp_sb = wpool.tile([P, KT, BL], dt.float32, tag="hp")
            hpT_sb = wpool.tile([BL, H], dt.float32, tag="hpT")
            ident4 = wpool.tile([BL, BL], dt.float32, tag="ident4")
            from concourse.masks import make_identity
            make_identity(nc, ident4[:])

            # hp on PE: hpT[b, k] = sum_h htT[h, b] * waT[h, k], then an
            # identity-matmul transpose back to [k partitions, (kt, b)].
            def emit_hp():
                for kc in range(TC):
                    pp = pbc.tile([BL, NT], dt.float32, tag="bc",
                                  name=f"hp_ps{kc}")
                    for ht in range(HT):
                        nc.tensor.matmul(
                            pp[:], htT_sb[:, ht, :], waT_kc[kc][:, ht, :],
                            start=(ht == 0), stop=(ht == HT - 1))
                    nc.vector.tensor_copy(
                        hpT_sb[:, kc * NT:(kc + 1) * NT], pp[:])
                hpt_ps = pbc.tile([P, KT * BL], dt.float32, tag="bc")
                for kt in range(KT):
                    nc.tensor.matmul(
                        hpt_ps[:, kt * BL:(kt + 1) * BL],
                        hpT_sb[:, kt * P:(kt + 1) * P], ident4[:],
                        start=True, stop=True)
                nc.vector.tensor_copy(
                    hp_sb[:].rearrange("p kt b -> p (kt b)"), hpt_ps[:])

            def make_tail(bi, ex, ssum, ctxp):
                # pre: 1/sum chain + attn output (DVE ops that must precede
                # the ctx1 STT batch in the DVE queue so the PE's rb matmul
                # is not blocked behind them).
                rinv_bf = smpool.tile([1, 1], dt.bfloat16, tag="rinv_bf",
                                      bufs=2, name=f"rinv_bf{bi}")

                def emit_pre():
                    stot = smpool.tile([1, 1], dt.float32, tag="stot", bufs=2)
                    nc.vector.tensor_reduce(stot[:], ssum[:],
                                            axis=mybir.AxisListType.X,
                                            op=ALU.add)
                    rinv = smpool.tile([1, 1], dt.float32, tag="rinv", bufs=2)
                    nc.vector.reciprocal(rinv[:], stot[:])
                    attn_sb = smpool.tile([1, T], dt.float32, tag="attn", bufs=2)
                    nc.vector.tensor_scalar_mul(attn_sb[:], ex[:], rinv[:])
                    nc.sync.dma_start(attn_d[bi:bi + 1, :], attn_sb[:])
                    nc.vector.tensor_copy(rinv_bf[:], rinv[:])

                def emit_post():
                    rb_ps = pbc.tile([P, 1], dt.float32, tag="bc")
                    nc.tensor.matmul(rb_ps[:], onesrow_bf[:], rinv_bf[:],
                                     start=True, stop=True)
                    rb_sb = smpool.tile([P, 1], dt.float32, tag="rb", bufs=2)
                    nc.vector.tensor_copy(rb_sb[:], rb_ps[:])
                    ctxs = smpool.tile([P, HT], dt.float32, tag="ctxs", bufs=2)
                    nc.vector.scalar_tensor_tensor(
                        ctxs[:], ctxp[0][:], 1.0, ctxp[1][:],
                        op0=ALU.mult, op1=ALU.add)
                    ctxc = smpool.tile([P, HT], dt.float32, tag="ctxc", bufs=2)
                    nc.vector.tensor_scalar_mul(ctxc[:], ctxs[:], rb_sb[:])
                    nc.sync.dma_start(ctx_d[bi], ctxc[:])
                return emit_pre, emit_post

            pending_tail = None
            pending_chunk1 = None
            pending_ctx1 = None
            encT_b1 = [None, None]

            def b0_prefetch():
                encT_b1[0] = load_encT(1, 0)
                encT_b1[1] = load_encT(1, 1)

            for bi in range(BL):
                if bi == 0:
                    encT_t = encT_b0
                elif bi == 1:
                    encT_t = encT_b1
                else:
                    encT_t = [load_encT(bi, tcc) for tcc in range(TC)]
                mask_f = smpool.tile([1, T], dt.float32, tag="mask", bufs=2)
                nc.gpsimd.dma_start(mask_f[:], mask_d[bi:bi + 1, :])
                mask_m1 = smpool.tile([1, T], dt.bfloat16, tag="mask_m1", bufs=2)
                nc.scalar.activation(mask_m1[:], mask_f[:], AF.Identity,
                                     bias=negbig[:], scale=1e30)

                ex = smpool.tile([1, T], dt.bfloat16, tag="ex", bufs=2)
                ssum = smpool.tile([1, TC], dt.float32, tag="ssum", bufs=2)
                ctxp = [smpool.tile([P, HT], dt.float32, tag=f"ctxp{_}",
                                    bufs=2, name=f"ctxp{_}")
                        for _ in range(TC)]
                chunk_th = [[], []]
                deferred_finish = []

                def make_chunk_run(tcc, ths, bi=bi, ex=ex, ssum=ssum,
                                   mask_m1=mask_m1):
                    # 8 back-to-back e-reduce matmuls (pipeline at full rate),
                    # then the mask add and the exp straight off PSUM.
                    def run():
                        e_ps = pe_ps.tile([1, NT], dt.float32, tag="e",
                                          name=f"e_ps{bi}_{tcc}")
                        for kt, th in ths:
                            nc.tensor.matmul(
                                e_ps[:], va_sb[:, kt:kt + 1], th[:],
                                start=(kt == 0), stop=False)
                        nc.tensor.matmul(
                            e_ps[:], ones_bf[:],
                            mask_m1[:, tcc * NT:(tcc + 1) * NT],
                            start=False, stop=True)
                        nc.scalar.activation(
                            ex[:, tcc * NT:(tcc + 1) * NT], e_ps[:],
                            AF.Exp, accum_out=ssum[:, tcc:tcc + 1])
                    return run

                def make_ctx_partial(tcc, bi=bi, ex=ex, ctxp=ctxp,
                                     encT_t=encT_t):
                    # broadcast this chunk's raw exp across partitions (PE),
                    # then reduce attn*encT per h-tile on the DVE.
                    def run():
                        exb = pbc.tile([P, NT], dt.float32, tag="bc",
                                       name=f"exb{bi}_{tcc}")
                        nc.tensor.matmul(
                            exb[:], onesrow_bf[:],
                            ex[:, tcc * NT:(tcc + 1) * NT],
                            start=True, stop=True)
                        for ht in range(HT):
                            scr = scpool.tile([P, NT], dt.bfloat16, tag="scr")
                            nc.vector.scalar_tensor_tensor(
                                scr[:], encT_t[tcc][:, ht, :], 1.0, exb[:],
                                op0=ALU.mult, op1=ALU.mult,
                                accum_out=ctxp[tcc][:, ht:ht + 1])
                    return run

                def finish_group(ps, kt, tcc):
                    th = thpool.tile([P, NT], dt.bfloat16, tag="th", name="th")
                    nc.scalar.activation(th[:], ps[:], AF.Tanh,
                                         bias=hp_sb[:, kt, bi:bi + 1])
                    chunk_th[tcc].append((kt, th))

                group_iter = [(kt, tcc) for tcc in range(TC)
                              for kt in range(KT)]
                hp_at = 3 if bi == 0 else -1
                gidx = 0
                for kt, tcc in group_iter:
                    if gidx == hp_at:
                        emit_hp()
                    if bi == 0 and gidx == 1:
                        b0_prefetch()
                    if gidx == 1 and pending_chunk1 is not None:
                        pending_chunk1()
                        pending_chunk1 = None
                    if gidx == 3 and pending_ctx1 is not None:
                        pending_tail[0]()          # rinv chain first (DVE)
                        pending_ctx1()
                        pending_ctx1 = None
                    if gidx == 4 and pending_tail is not None:
                        pending_tail[1]()
                        pending_tail = None
                    if gidx == KT + 1:
                        make_chunk_run(0, chunk_th[0])()
                    if gidx == KT + 3:
                        make_ctx_partial(0)()
                    ps = pmain.tile([P, NT], dt.float32, tag="big")
                    for ht in range(HT):
                        nc.tensor.matmul(
                            ps[:], uaT_sb[kt][:, ht, :],
                            encT_t[tcc][:, ht, :],
                            start=(ht == 0), stop=(ht == HT - 1))
                    if bi == 0 and gidx < hp_at:
                        deferred_finish.append((ps, kt, tcc))
                    else:
                        if deferred_finish:
                            for args in deferred_finish:
                                finish_group(*args)
                            deferred_finish = []
                        finish_group(ps, kt, tcc)
                    gidx += 1
                if bi == BL - 1:
                    make_chunk_run(1, chunk_th[1])()
                    pre, post = make_tail(bi, ex, ssum, ctxp)
                    pre()
                    make_ctx_partial(1)()
                    post()
                else:
                    pending_chunk1 = make_chunk_run(1, chunk_th[1])
                    pending_ctx1 = make_ctx_partial(1)
                    pending_tail = make_tail(bi, ex, ssum, ctxp)

    nc.compile()
    return nc


def _get_runner():
    if "runner" in _CACHE:
        return _CACHE["runner"]

    import jax
    from jax.sharding import Mesh, PartitionSpec
    from jax.experimental.shard_map import shard_map
    from concourse import bass2jax
    from concourse import mybir as _mb

    nc = _build_nc()
    bass2jax.install_neuronx_cc_hook()

    partition_name = (nc.partition_id_tensor.name
                      if nc.partition_id_tensor else None)
    in_names, out_names, out_avals, zero_outs = [], [], [], []
    for alloc in nc.m.functions[0].allocations:
        if not isinstance(alloc, _mb.MemoryLocationSet):
            continue
        name = alloc.memorylocations[0].name
        if alloc.kind == "ExternalInput":
            if name != partition_name:
                in_names.append(name)
        elif alloc.kind == "ExternalOutput":
            out_names.append(name)
            shape = tuple(alloc.tensor_shape)
            npdt = _mb.dt.np(alloc.dtype)
            out_avals.append(jax.core.ShapedArray(shape, npdt))
            zero_outs.append(np.zeros(shape, npdt))
    n_params = len(in_names)
    n_outs = len(out_names)
    all_in_names = in_names + out_names
    if partition_name is not None:
        all_in_names = all_in_names + [partition_name]
    donate = tuple(range(n_params, n_params + n_outs))

    def _body(*args):
        operands = list(args)
        if partition_name is not None:
            operands.append(bass2jax.partition_id_tensor())
        outs = bass2jax._bass_exec_p.bind(
            *operands,
            out_avals=tuple(out_avals),
            in_names=tuple(all_in_names),
            out_names=tuple(out_names),
            lowering_input_output_aliases=(),
            sim_require_finite=True,
            sim_require_nnan=True,
            nc=nc,
        )
        return tuple(outs)

    devices = jax.devices()[:NCORES]
    mesh = Mesh(np.asarray(devices), ("core",))
    in_specs = (PartitionSpec("core"),) * (n_params + n_outs)
    out_specs = (PartitionSpec("core"),) * n_outs
    sharded = jax.jit(
        shard_map(_body, mesh=mesh, in_specs=in_specs, out_specs=out_specs,
                  check_rep=False),
        donate_argnums=donate, keep_unused=True)

    def run(in_maps):
        concat_in = [
            np.concatenate([np.asarray(m[name]) for m in in_maps], axis=0)
            for name in in_names
        ]
        concat_zeros = [
            np.zeros((NCORES * z.shape[0], *z.shape[1:]), z.dtype)
            for z in zero_outs
        ]
        out_arrs = sharded(*concat_in, *concat_zeros)
        return [
            {name: np.asarray(out_arrs[i]).reshape(NCORES, *out_avals[i].shape)[c]
             for i, name in enumerate(out_names)}
            for c in range(NCORES)
        ]

    _CACHE["runner"] = run
    return run


def _make_in_maps(inputs):
    import ml_dtypes
    bf16 = ml_dtypes.bfloat16

    h_t = np.asarray(inputs["h_t"], dtype=np.float32)
    enc_out = np.asarray(inputs["enc_out"], dtype=np.float32)
    src_mask = np.asarray(inputs["src_mask"])
    Wa = np.asarray(inputs["Wa"], dtype=np.float32)
    Ua = np.asarray(inputs["Ua"], dtype=np.float32)
    va = np.asarray(inputs["va"], dtype=np.float32)

    uaT = np.ascontiguousarray(
        Ua.T.reshape(HT, P, KT, P).transpose(2, 1, 0, 3)).astype(bf16)
    waT = np.ascontiguousarray(
        Wa.T.reshape(HT, P, TC, NT).transpose(2, 1, 0, 3)).astype(bf16)
    htT = np.ascontiguousarray(h_t.T).astype(bf16)               # [H, B]
    encT = np.ascontiguousarray(
        enc_out.transpose(0, 2, 1).reshape(B, HT, P, TC, NT)
        .transpose(0, 3, 2, 1, 4)).astype(bf16)                  # [B, TC, P, HT, NT]
    mask_u8 = np.ascontiguousarray(src_mask.astype(np.uint8))

    in_maps = []
    for c in range(NCORES):
        sl = slice(c * BL, (c + 1) * BL)
        in_maps.append({
            "encT": encT[sl],
            "uaT": uaT,
            "waT": waT,
            "htT": np.ascontiguousarray(htT[:, sl]),
            "va": va.astype(bf16),
            "mask": mask_u8[sl],
        })
    return in_maps


def kernel(h_t, enc_out, src_mask, Wa, Ua, va):
    in_maps = _make_in_maps({
        "h_t": h_t, "enc_out": enc_out, "src_mask": src_mask,
        "Wa": Wa, "Ua": Ua, "va": va,
    })
    run = _get_runner()
    results = run(in_maps)
    context = np.concatenate(
        [r["ctx"].transpose(0, 2, 1).reshape(BL, H) for r in results], axis=0)
    attn = np.concatenate([r["attn"] for r in results], axis=0)
    return context, attn


# revision 14
# speedup vs baseline: 1.0258x; 1.0258x over previous
"""Bahdanau additive attention on 8 Trainium2 NeuronCores.

Data-parallel over batch: core c handles batches [4c, 4c+4).
Per batch b (all big PE operands bf16; PSUM accumulates fp32):
  ep[k,t]   = sum_h Ua[k,h] * enc[b,t,h]        (PE matmuls, k on PSUM partitions)
  z[k,t]    = tanh(ep[k,t] + hp[b,k])           (ScalarE, hp as per-partition bias)
  e[t]      = sum_k va[k] * z[k,t]              (M=1 PE matmuls + mask add)
  attn      = exp(e) / sum(exp(e))              (per-chunk exp straight from PSUM;
                                                 no max pass: |e| <= sum|va| ~ 26;
                                                 mask folds in as -1e30 -> exp -> 0)
  ctx[h]    = sum_t attn[t] * enc[b,t,h]        (DVE tensor_tensor_reduce against a
                                                 PE partition-broadcast of attn,
                                                 reusing the already-resident encT
                                                 tiles -- no second enc load)
hp[b,k] = sum_h Wa[k,h] * h_t[b,h] on the PE via transposed-layout matmuls.
Prologue DMAs fan out over three queues (sync/gpsimd/vector) and the first
tiles are split per h-slice so the first matmul starts ~1.5us in.
"""

import numpy as np

import concourse.bass as bass
import concourse.tile as tile
from concourse import bacc, mybir

dt = mybir.dt
AF = mybir.ActivationFunctionType
ALU = mybir.AluOpType

B, T, H = 32, 1024, 1024
NCORES = 8
BL = B // NCORES          # batches per core
P = 128                   # partitions
NT = 512                  # matmul free-dim chunk (one PSUM bank of fp32)
KT = H // P               # k-tiles (output rows of ep)
HT = H // P               # h-tiles (contraction)
TT = T // P               # t-tiles
TC = T // NT              # t chunks per batch

_CACHE = {}


def _build_nc():
    nc = bacc.Bacc("TRN2", target_bir_lowering=False, debug=False)

    # Block layouts (host-prepped) so each DMA is one contiguous transfer
    # delivering exactly what one matmul group consumes:
    #   encT: [BL, TC, H, NT]  tc-major blocks of enc[b].T
    #   uaT:  [KT, H, P]       kt-major column blocks of Ua.T
    #   waT:  [TC, H, NT]      kc-major column blocks of Wa.T
    encT_d = nc.dram_tensor("encT", [BL, TC, P, HT, NT], dt.bfloat16,
                            kind="ExternalInput").ap()
    uaT_d = nc.dram_tensor("uaT", [KT, P, HT, P], dt.bfloat16,
                           kind="ExternalInput").ap()
    waT_d = nc.dram_tensor("waT", [TC, P, HT, NT], dt.bfloat16,
                           kind="ExternalInput").ap()
    htT_d = nc.dram_tensor("htT", [H, BL], dt.bfloat16, kind="ExternalInput").ap()
    va_d = nc.dram_tensor("va", [H], dt.bfloat16, kind="ExternalInput").ap()
    mask_d = nc.dram_tensor("mask", [BL, T], dt.uint8, kind="ExternalInput").ap()

    # ctx stored [BL, P, HT] (partition-major) so the output DMA is one
    # contiguous 2D transfer; the host undoes the permutation.
    ctx_d = nc.dram_tensor("ctx", [BL, P, HT], dt.float32,
                           kind="ExternalOutput").ap()
    attn_d = nc.dram_tensor("attn", [BL, T], dt.float32, kind="ExternalOutput").ap()

    with tile.TileContext(nc) as tc:
        from contextlib import ExitStack

        with ExitStack() as st:
            wpool = st.enter_context(tc.tile_pool(name="weights", bufs=1))
            etpool = st.enter_context(tc.tile_pool(name="encT", bufs=4))
            b0pool = st.enter_context(tc.tile_pool(name="b0enc", bufs=HT))
            thpool = st.enter_context(tc.tile_pool(name="tanh", bufs=12))
            scpool = st.enter_context(tc.tile_pool(name="scr", bufs=2))
            smpool = st.enter_context(tc.tile_pool(name="small", bufs=1))
            pmain = st.enter_context(tc.tile_pool(name="pmain", bufs=5, space="PSUM"))
            pe_ps = st.enter_context(tc.tile_pool(name="pe", bufs=2, space="PSUM"))
            pbc = st.enter_context(tc.tile_pool(name="pbc", bufs=1, space="PSUM"))

            # ---- constants / small inputs, fanned across DMA queues ----
            va_sb = wpool.tile([P, KT], dt.bfloat16, tag="va")
            htT_sb = wpool.tile([P, HT, BL], dt.bfloat16, tag="htT")
            nc.gpsimd.dma_start(htT_sb[:], htT_d.rearrange("(ht p) b -> p ht b", p=P))

            uaT_sb = [None] * KT

            def load_uaT(kt):
                u = wpool.tile([P, HT, P], dt.bfloat16, tag=f"uaT{kt}",
                               name=f"uaT{kt}")
                nc.scalar.dma_start(u[:], uaT_d[kt])
                uaT_sb[kt] = u

            def load_encT(bi, tcc):
                t_ = etpool.tile([P, HT, NT], dt.bfloat16, tag="encT",
                                 name=f"encT{bi}_{tcc}")
                nc.sync.dma_start(t_[:], encT_d[bi, tcc])
                return [t_[:, ht, :] for ht in range(HT)]

            def load_waT(kc):
                w = wpool.tile([P, HT, NT], dt.bfloat16, tag=f"waT{kc}",
                               name=f"waT{kc}")
                hh = HT // 2
                nc.gpsimd.dma_start(w[:, :hh, :], waT_d[kc, :, :hh, :])
                nc.gpsimd.dma_start(w[:, hh:, :], waT_d[kc, :, hh:, :])
                return w

            # Reads wait on a tile's FULL set of writes, so the first
            # chunk's h-slices live in 8 standalone tiles, round-robined over
            # the sync+gpsimd queues -- each main matmul of group 0 depends on
            # exactly one small DMA. uaT goes first on the scalar queue.
            for kt in range(KT):
                load_uaT(kt)
            nc.scalar.dma_start(va_sb[:], va_d.rearrange("(kt p) -> p kt", p=P))
            b0c0 = []
            eng_ring = [nc.sync, nc.gpsimd]
            for ht in range(HT):
                s = b0pool.tile([P, NT], dt.bfloat16, tag="b0enc",
                                name=f"b0enc{ht}")
                eng_ring[ht % 2].dma_start(s[:], encT_d[0, 0, :, ht, :])
                b0c0.append(s[:])
            encT_b0 = [b0c0]
            waT_kc = [load_waT(0), load_waT(1)]
            encT_b0.append(load_encT(0, 1))

            ones_sb = wpool.tile([1, 1], dt.float32, tag="ones")
            nc.vector.memset(ones_sb[:], 1.0)
            ones_bf = wpool.tile([1, 1], dt.bfloat16, tag="ones_bf")
            nc.vector.tensor_copy(ones_bf[:], ones_sb[:])
            onesrow_sb = wpool.tile([1, P], dt.float32, tag="onesrow")
            nc.vector.memset(onesrow_sb[:], 1.0)
            onesrow_bf = wpool.tile([1, P], dt.bfloat16, tag="onesrow_bf")
            nc.vector.tensor_copy(onesrow_bf[:], onesrow_sb[:])
            negbig = wpool.tile([1, 1], dt.float32, tag="negbig")
            nc.vector.memset(negbig[:], -1e30)
            hp_sb = wpool.tile([P, KT, BL], dt.float32, tag="hp")
            hpT_sb = wpool.tile([BL, H], dt.float32, tag="hpT")
            ident4 = wpool.tile([BL, BL], dt.float32, tag="ident4")
            from concourse.masks import make_identity
            make_identity(nc, ident4[:])

            # hp on PE: hpT[b, k] = sum_h htT[h, b] * waT[h, k], then an
            # identity-matmul transpose back to [k partitions, (kt, b)].
            def emit_hp():
                for kc in range(TC):
                    pp = pbc.tile([BL, NT], dt.float32, tag="bc",
                                  name=f"hp_ps{kc}")
                    for ht in range(HT):
                        nc.tensor.matmul(
                            pp[:], htT_sb[:, ht, :], waT_kc[kc][:, ht, :],
                            start=(ht == 0), stop=(ht == HT - 1))
                    nc.vector.tensor_copy(
                        hpT_sb[:, kc * NT:(kc + 1) * NT], pp[:])
                hpt_ps = pbc.tile([P, KT * BL], dt.float32, tag="bc")
                for kt in range(KT):
                    nc.tensor.matmul(
                        hpt_ps[:, kt * BL:(kt + 1) * BL],
                        hpT_sb[:, kt * P:(kt + 1) * P], ident4[:],
                        start=True, stop=True)
                nc.vector.tensor_copy(
                    hp_sb[:].rearrange("p kt b -> p (kt b)"), hpt_ps[:])

            def make_tail(bi, ex, ssum, ctxp):
                # pre: 1/sum chain + attn output (DVE ops that must precede
                # the ctx1 STT batch in the DVE queue so the PE's rb matmul
                # is not blocked behind them).
                rinv_bf = smpool.tile([1, 1], dt.bfloat16, tag="rinv_bf",
                                      bufs=2, name=f"rinv_bf{bi}")

                def emit_pre():
                    stot = smpool.tile([1, 1], dt.float32, tag="stot", bufs=2)
                    nc.vector.tensor_reduce(stot[:], ssum[:],
                                            axis=mybir.AxisListType.X,
                                            op=ALU.add)
                    rinv = smpool.tile([1, 1], dt.float32, tag="rinv", bufs=2)
                    nc.vector.reciprocal(rinv[:], stot[:])
                    attn_sb = smpool.tile([1, T], dt.float32, tag="attn", bufs=2)
                    nc.vector.tensor_scalar_mul(attn_sb[:], ex[:], rinv[:])
                    nc.sync.dma_start(attn_d[bi:bi + 1, :], attn_sb[:])
                    nc.vector.tensor_copy(rinv_bf[:], rinv[:])

                def emit_post():
                    rb_ps = pe_ps.tile([P, 1], dt.float32, tag="e")
                    nc.tensor.matmul(rb_ps[:], onesrow_bf[:], rinv_bf[:],
                                     start=True, stop=True)
                    rb_sb = smpool.tile([P, 1], dt.float32, tag="rb", bufs=2)
                    nc.vector.tensor_copy(rb_sb[:], rb_ps[:])
                    ctxs = smpool.tile([P, HT], dt.float32, tag="ctxs", bufs=2)
                    nc.vector.scalar_tensor_tensor(
                        ctxs[:], ctxp[0][:], 1.0, ctxp[1][:],
                        op0=ALU.mult, op1=ALU.add)
                    ctxc = smpool.tile([P, HT], dt.float32, tag="ctxc", bufs=2)
                    nc.vector.tensor_scalar_mul(ctxc[:], ctxs[:], rb_sb[:])
                    nc.sync.dma_start(ctx_d[bi], ctxc[:])
                return emit_pre, emit_post

            pending_tail = None
            pending_chunk1 = None
            pending_ctx1 = None
            encT_b1 = [None, None]

            def b0_prefetch():
                encT_b1[0] = load_encT(1, 0)
                encT_b1[1] = load_encT(1, 1)

            for bi in range(BL):
                if bi == 0:
                    encT_t = encT_b0
                elif bi == 1:
                    encT_t = encT_b1
                else:
                    encT_t = [load_encT(bi, tcc) for tcc in range(TC)]
                mask_f = smpool.tile([1, T], dt.float32, tag="mask", bufs=2)
                nc.gpsimd.dma_start(mask_f[:], mask_d[bi:bi + 1, :])
                mask_m1 = smpool.tile([1, T], dt.bfloat16, tag="mask_m1", bufs=2)
                nc.scalar.activation(mask_m1[:], mask_f[:], AF.Identity,
                                     bias=negbig[:], scale=1e30)

                ex = smpool.tile([1, T], dt.bfloat16, tag="ex", bufs=2)
                ssum = smpool.tile([1, TC], dt.float32, tag="ssum", bufs=2)
                ctxp = [smpool.tile([P, HT], dt.float32, tag=f"ctxp{_}",
                                    bufs=2, name=f"ctxp{_}")
                        for _ in range(TC)]
                chunk_th = [[], []]
                deferred_finish = []

                def make_chunk_run(tcc, ths, bi=bi, ex=ex, ssum=ssum,
                                   mask_m1=mask_m1):
                    # 8 back-to-back e-reduce matmuls (pipeline at full rate),
                    # then the mask add and the exp straight off PSUM.
                    def run():
                        e_ps = pe_ps.tile([1, NT], dt.float32, tag="e",
                                          name=f"e_ps{bi}_{tcc}")
                        for kt, th in ths:
                            nc.tensor.matmul(
                                e_ps[:], va_sb[:, kt:kt + 1], th[:],
                                start=(kt == 0), stop=False)
                        nc.tensor.matmul(
                            e_ps[:], ones_bf[:],
                            mask_m1[:, tcc * NT:(tcc + 1) * NT],
                            start=False, stop=True)
                        nc.scalar.activation(
                            ex[:, tcc * NT:(tcc + 1) * NT], e_ps[:],
                            AF.Exp, accum_out=ssum[:, tcc:tcc + 1])
                    return run

                def make_ctx_partial(tcc, bi=bi, ex=ex, ctxp=ctxp,
                                     encT_t=encT_t):
                    # broadcast this chunk's raw exp across partitions (PE),
                    # then reduce attn*encT per h-tile on the DVE.
                    def run():
                        exb = pbc.tile([P, NT], dt.float32, tag="bc",
                                       name=f"exb{bi}_{tcc}")
                        nc.tensor.matmul(
                            exb[:], onesrow_bf[:],
                            ex[:, tcc * NT:(tcc + 1) * NT],
                            start=True, stop=True)
                        for ht in range(HT):
                            scr = scpool.tile([P, NT], dt.bfloat16, tag="scr")
                            nc.vector.scalar_tensor_tensor(
                                scr[:], encT_t[tcc][ht], 1.0, exb[:],
                                op0=ALU.mult, op1=ALU.mult,
                                accum_out=ctxp[tcc][:, ht:ht + 1])
                    return run

                def finish_group(ps, kt, tcc):
                    th = thpool.tile([P, NT], dt.bfloat16, tag="th", name="th")
                    nc.scalar.activation(th[:], ps[:], AF.Tanh,
                                         bias=hp_sb[:, kt, bi:bi + 1])
                    chunk_th[tcc].append((kt, th))

                group_iter = [(kt, tcc) for tcc in range(TC)
                              for kt in range(KT)]
                hp_at = 4 if bi == 0 else -1
                gidx = 0
                for kt, tcc in group_iter:
                    if gidx == hp_at:
                        emit_hp()
                    if bi == 0 and gidx == 1:
                        b0_prefetch()
                    if gidx == 1 and pending_chunk1 is not None:
                        pending_chunk1()
                        pending_chunk1 = None
                    if gidx == 3 and pending_ctx1 is not None:
                        pending_tail[0]()          # rinv chain first (DVE)
                        pending_ctx1()
                        pending_ctx1 = None
                    if gidx == 4 and pending_tail is not None:
                        pending_tail[1]()
                        pending_tail = None
                    if gidx == KT + 1:
                        make_chunk_run(0, chunk_th[0])()
                    if gidx == KT + 3:
                        make_ctx_partial(0)()
                    ps = pmain.tile([P, NT], dt.float32, tag="big")
                    for ht in range(HT):
                        nc.tensor.matmul(
                            ps[:], uaT_sb[kt][:, ht, :],
                            encT_t[tcc][ht],
                            start=(ht == 0), stop=(ht == HT - 1))
                    if bi == 0 and gidx < hp_at:
                        deferred_finish.append((ps, kt, tcc))
                    else:
                        if deferred_finish:
                            for args in deferred_finish:
                                finish_group(*args)
                            deferred_finish = []
                        finish_group(ps, kt, tcc)
                    gidx += 1
                if bi == BL - 1:
                    make_chunk_run(1, chunk_th[1])()
                    pre, post = make_tail(bi, ex, ssum, ctxp)
                    pre()
                    make_ctx_partial(1)()
                    post()
                else:
                    pending_chunk1 = make_chunk_run(1, chunk_th[1])
                    pending_ctx1 = make_ctx_partial(1)
                    pending_tail = make_tail(bi, ex, ssum, ctxp)

    nc.compile()
    return nc


def _get_runner():
    if "runner" in _CACHE:
        return _CACHE["runner"]

    import jax
    from jax.sharding import Mesh, PartitionSpec
    from jax.experimental.shard_map import shard_map
    from concourse import bass2jax
    from concourse import mybir as _mb

    nc = _build_nc()
    bass2jax.install_neuronx_cc_hook()

    partition_name = (nc.partition_id_tensor.name
                      if nc.partition_id_tensor else None)
    in_names, out_names, out_avals, zero_outs = [], [], [], []
    for alloc in nc.m.functions[0].allocations:
        if not isinstance(alloc, _mb.MemoryLocationSet):
            continue
        name = alloc.memorylocations[0].name
        if alloc.kind == "ExternalInput":
            if name != partition_name:
                in_names.append(name)
        elif alloc.kind == "ExternalOutput":
            out_names.append(name)
            shape = tuple(alloc.tensor_shape)
            npdt = _mb.dt.np(alloc.dtype)
            out_avals.append(jax.core.ShapedArray(shape, npdt))
            zero_outs.append(np.zeros(shape, npdt))
    n_params = len(in_names)
    n_outs = len(out_names)
    all_in_names = in_names + out_names
    if partition_name is not None:
        all_in_names = all_in_names + [partition_name]
    donate = tuple(range(n_params, n_params + n_outs))

    def _body(*args):
        operands = list(args)
        if partition_name is not None:
            operands.append(bass2jax.partition_id_tensor())
        outs = bass2jax._bass_exec_p.bind(
            *operands,
            out_avals=tuple(out_avals),
            in_names=tuple(all_in_names),
            out_names=tuple(out_names),
            lowering_input_output_aliases=(),
            sim_require_finite=True,
            sim_require_nnan=True,
            nc=nc,
        )
        return tuple(outs)

    devices = jax.devices()[:NCORES]
    mesh = Mesh(np.asarray(devices), ("core",))
    in_specs = (PartitionSpec("core"),) * (n_params + n_outs)
    out_specs = (PartitionSpec("core"),) * n_outs
    sharded = jax.jit(
        shard_map(_body, mesh=mesh, in_specs=in_specs, out_specs=out_specs,
                  check_rep=False),
        donate_argnums=donate, keep_unused=True)

    def run(in_maps):
        concat_in = [
            np.concatenate([np.asarray(m[name]) for m in in_maps], axis=0)
            for name in in_names
        ]
        concat_zeros = [
            np.zeros((NCORES * z.shape[0], *z.shape[1:]), z.dtype)
            for z in zero_outs
        ]
        out_arrs = sharded(*concat_in, *concat_zeros)
        return [
            {name: np.asarray(out_arrs[i]).reshape(NCORES, *out_avals[i].shape)[c]
             for i, name in enumerate(out_names)}
            for c in range(NCORES)
        ]

    _CACHE["runner"] = run
    return run


def _make_in_maps(inputs):
    import ml_dtypes
    bf16 = ml_dtypes.bfloat16

    h_t = np.asarray(inputs["h_t"], dtype=np.float32)
    enc_out = np.asarray(inputs["enc_out"], dtype=np.float32)
    src_mask = np.asarray(inputs["src_mask"])
    Wa = np.asarray(inputs["Wa"], dtype=np.float32)
    Ua = np.asarray(inputs["Ua"], dtype=np.float32)
    va = np.asarray(inputs["va"], dtype=np.float32)

    uaT = np.ascontiguousarray(
        Ua.T.reshape(HT, P, KT, P).transpose(2, 1, 0, 3)).astype(bf16)
    waT = np.ascontiguousarray(
        Wa.T.reshape(HT, P, TC, NT).transpose(2, 1, 0, 3)).astype(bf16)
    htT = np.ascontiguousarray(h_t.T).astype(bf16)               # [H, B]
    encT = np.ascontiguousarray(
        enc_out.transpose(0, 2, 1).reshape(B, HT, P, TC, NT)
        .transpose(0, 3, 2, 1, 4)).astype(bf16)                  # [B, TC, P, HT, NT]
    mask_u8 = np.ascontiguousarray(src_mask.astype(np.uint8))

    in_maps = []
    for c in range(NCORES):
        sl = slice(c * BL, (c + 1) * BL)
        in_maps.append({
            "encT": encT[sl],
            "uaT": uaT,
            "waT": waT,
            "htT": np.ascontiguousarray(htT[:, sl]),
            "va": va.astype(bf16),
            "mask": mask_u8[sl],
        })
    return in_maps


def kernel(h_t, enc_out, src_mask, Wa, Ua, va):
    in_maps = _make_in_maps({
        "h_t": h_t, "enc_out": enc_out, "src_mask": src_mask,
        "Wa": Wa, "Ua": Ua, "va": va,
    })
    run = _get_runner()
    results = run(in_maps)
    context = np.concatenate(
        [r["ctx"].transpose(0, 2, 1).reshape(BL, H) for r in results], axis=0)
    attn = np.concatenate([r["attn"] for r in results], axis=0)
    return context, attn
